# revision 46
# baseline (speedup 1.0000x reference)
"""Causal single-head attention on 8 Trainium2 NeuronCores.

Problem: x[8, 4096, 512] @ W_{Q,K,V}[512, 64] -> causal softmax attention
-> out[8, 4096, 64].

Sharding: data-parallel over batch, one batch element per core (B == n_cores
== 8), QKV weights replicated. No collectives.

Per-core design (S=4096, D=512, E=64):
  - x is staged host-side as bf16 and enters SBUF PRE-TRANSPOSED through the
    DMA xbar transpose engine (dma_start_transpose), one [512,512] chunk at
    a time: the PE never spends a cycle transposing x (the previous design
    burned ~25k PE cycles + 14k DVE cycles on that).
  - Transposed score layout ST[k_par, q_free]; the softmax denominator
    falls out of the PV matmul via an appended ones-column on V
    (v_aug [k, 65] in a stride-66 bf16 layout -> row 64 of out.T
    accumulates sum_k P[k,q]).
  - bf16 operands for every matmul (1 PE cycle/row at ANY moving width,
    unlike f32r's N>=256 requirement); fp32 PSUM accumulation.
  - Scores matmuls contract over E=64, so two k-tiles are packed into
    the PE array quadrants (tile_position (0,0)/(64,0)) and run concurrently.
  - Softmax exp is split across TWO engines: ACT (table exp, bf16 out) and
    DVE (Schraudolph bit-trick: i16 = s*(128/ln2)/8 + B via one tensor_scalar
    with int16 output; the int16 bit pattern IS bf16 exp(s/8) to within
    +-3%). Offloaded pairs are full (non-diagonal) so no masking is needed
    and every PSUM element is freshly written; 1/3 of full pairs go to DVE.
    Accuracy: 6.3e-3 rel err end-to-end on HW (gate 2e-2); exp is
    unnormalized (no max subtraction; |s|/8 <= ~6).
  - vT->v and the epilogue outT->out transposes also go through the DMA
    xbar instead of the PE, issued on the ACT HWDGE ring (nc.scalar) so they
    never queue behind the 2us x chunk loads on the SP ring (doing them on
    the SP ring measured 13us SLOWER). The xbar needs contiguous dests:
    v stages through a [128,4,64] tile + 4x-mode DVE copy; the epilogue
    transposes a fixed [80,512] tile (rows 65..79 zero-padded once).
  - Causality: strictly-upper tile pairs are skipped, diagonal-crossing
    tiles get column-restricted matmuls/exp plus a GpSimd affine_select
    zeroing the 128x128 triangle of exp'd scores.

Measured (reps-slope estimator, median/trimmed-mean of 80 diffs):
  baseline(prev session) 138.4/145.0 us -> this design 115.5/103.6 us.
  The same estimator overstates the harness "HW exec time" by a per-rep
  pipeline-boundary cost (~10us on an empty kernel).

HW facts learned (probes in probe_pe.py / probe.py):
  - PE never reaches the 2.4GHz warm rate here: 128 back-to-back N=512 bf16
    matmuls measure 470ns each with per-matmul PSUM-slot rotation, 282ns in
    a single accumulation group (213ns would be warm); per-instruction
    semaphore/slot overhead ~70-190ns dominates, so FEWER/BIGGER PE
    instructions and DMA-xbar offload beat cycle-count micro-optimizations.
  - Column tiling with tile_position (0,64) (M=64) dies at runtime (xbus
    quadrant-3 HW bug), so the V projection stays un-packed and the
    4-way-packed denominator scheme is infeasible.
  - bf16 matmul->PSUM output is TRN3-only (bass assert + HW); N caps at 512
    fp32 (one PSUM bank) for every matmul.
  - x DMA streams at ~330GB/s (8MB in ~25us, fully overlapped).

Schedule (Tile scheduler is a greedy priority heap; priority == emission
order):
  - Inner loop software-pipelined TWO deep: scores_{j}/exp_{j} are emitted
    two pairs before PV_{j} so the in-order PE queue never blocks behind a
    PV that waits on an ACT/DVE exp.
  - PSUM: psst 3x[128,1024]=6 banks (scores/exp pipeline), pso 1 (PV
    accumulator), pspj 1 (projections; phase-split so prologue(c+1) never
    chains behind chunk c's attention).
  - Projections are emitted before the next chunk's x staging; epilogue(c)
    after prologue(c+1) so its out-DMA yields to x prefetches.
  - A 1-element exp at build start pre-loads the ACT exp table set.
"""

import sys

sys.path.insert(0, "/opt/trn_rl_repo")
sys.path.insert(0, "/root/.axon_site/_ro/trn_rl_repo")

import numpy as np

B, S, D, E = 8, 4096, 512, 64
N_CORES = 8

# Schraudolph int16/bf16 exp: i16 = floor(z * 128/ln2 + B); bits are bf16.
# A includes the softmax 1/sqrt(64). B centers the sawtooth (+0.5 if HW
# truncates, which it does in CoreSim; the 0.25-unit ambiguity vs
# round-to-nearest is a 0.14% weight bias either way).
A_TRICK = float((128.0 / np.log(2.0)) / 8.0)
B_TRICK = 16250.65
# offload full pair j of chunk c to the DVE trick when j % MOD == PHASE
OFF_MOD, OFF_PHASE = 3, 1
V_PACK = False      # column-packed V projection (tile_position (0,64))
OFFLOAD = True      # DVE Schraudolph exp offload

_cache = {}


def _build(S=S, reps=1):
    import concourse.bass as bass
    import concourse.mybir as mybir
    import concourse.tile as tile
    from concourse import bacc
    from concourse.masks import make_identity

    F32 = mybir.dt.float32
    F32R = mybir.dt.float32r
    BF16 = mybir.dt.bfloat16
    I16 = mybir.dt.int16
    U16 = mybir.dt.uint16
    EXP = mybir.ActivationFunctionType.Exp
    MULT = mybir.AluOpType.mult
    ADD = mybir.AluOpType.add

    T = S // 128   # 128-row seq tiles
    C = S // 512   # 512-col q chunks
    DC = D // 128  # contraction chunks

    nc = bacc.Bacc("TRN2", target_bir_lowering=False, debug=False,
                   num_devices=N_CORES)
    xbf = nc.dram_tensor("x_bf16", [S, D], BF16, kind="ExternalInput").ap()
    wq = nc.dram_tensor("W_Q", [D, E], F32, kind="ExternalInput").ap()
    wk = nc.dram_tensor("W_K", [D, E], F32, kind="ExternalInput").ap()
    wv = nc.dram_tensor("W_V", [D, E], F32, kind="ExternalInput").ap()
    out = nc.dram_tensor("out", [S, E], F32, kind="ExternalOutput").ap()

    with tile.TileContext(nc) as tc:
        from contextlib import ExitStack

        with ExitStack() as ctx:
            const = ctx.enter_context(tc.tile_pool(name="const", bufs=1))
            big = ctx.enter_context(tc.tile_pool(name="big", bufs=1))
            sbw = ctx.enter_context(tc.tile_pool(name="work", bufs=4))
            ptp = ctx.enter_context(tc.tile_pool(name="pt", bufs=8))
            ptq = ctx.enter_context(tc.tile_pool(name="ptq", bufs=4))
            # PSUM budget (8 banks): psst 3x[128,1024 f32]=6, pso 1,
            # pspj 1 (projections). x arrives pre-transposed via the DMA
            # xbar so no transpose-staging banks are needed.
            pspj = ctx.enter_context(tc.tile_pool(name="pspj", bufs=1, space="PSUM"))
            psst = ctx.enter_context(tc.tile_pool(name="psst", bufs=3, space="PSUM"))
            pso = ctx.enter_context(tc.tile_pool(name="pso", bufs=1, space="PSUM"))

            # ---------------- constants ----------------
            wstage = const.tile([128, DC, 2 * E], F32)
            nc.sync.dma_start(wstage[:, :, 0:E], wk.rearrange("(c p) e -> p c e", p=128))
            nc.sync.dma_start(wstage[:, :, E:2 * E], wq.rearrange("(c p) e -> p c e", p=128))
            wvstage = const.tile([128, DC, E], F32)
            nc.sync.dma_start(wvstage[:], wv.rearrange("(c p) e -> p c e", p=128))
            # weights to bf16; out rows of QK psum: 0:64 = kT, 64:128 = qT
            wkq_t = const.tile([128, DC, 2 * E], BF16)
            nc.vector.tensor_copy(wkq_t[:], wstage[:])
            wv_t = const.tile([128, DC, E], BF16)
            nc.vector.tensor_copy(wv_t[:], wvstage[:])

            ident_bf = const.tile([128, 128], BF16)
            make_identity(nc, ident_bf[:])

            ones_st = const.tile([128, T], BF16)
            nc.gpsimd.memset(ones_st[:], 1.0)
            # warm the ACT exp table set before the first real exp
            warm = const.tile([1, 1], F32)
            nc.scalar.activation(warm[:], ident_bf[0:1, 0:1], EXP)

            # ---------------- big SBUF residents ----------------
            xT = big.tile([128, DC, S], BF16)        # x transposed, d on partitions
            qkALL = big.tile([128, S], BF16)         # [0:64]=kT, [64:128]=qT
            QLK = big.tile([128, S], BF16)           # [0:64]=qT ; [64:128, 0:S//2]=kT odd tiles
            # v rows + ones col; row stride padded to E+2 (bf16 4B-align)
            v_aug = big.tile([128, T, E + 2], BF16)
            nc.vector.tensor_copy(v_aug[:, :, E:E + 1], ones_st[:])
            # epilogue staging: fixed buffers (bufs=1 semantics) so the
            # one-time pad memset below stays valid across chunks
            ot_fix = big.tile([80, 512], BF16)       # out^T + den row, padded to 80
            nc.gpsimd.memset(ot_fix[E:80, :], 0.0)

            def stage_x(cc):
                # xT arrives pre-transposed straight from DRAM via the DMA
                # xbar (x is staged host-side as bf16). One transposing DMA
                # per chunk; zero PE/DVE/GPSIMD cost.
                nc.sync.dma_start_transpose(
                    xT[:, :, 512 * cc:512 * (cc + 1)],
                    xbf[512 * cc:512 * (cc + 1), :])

            def prologue(c, first=False):
                # x staging runs one chunk ahead (xT[c] was staged during
                # prologue(c-1)); projections are emitted FIRST since they
                # head the critical chain into this chunk's scores.
                if c == 0:
                    stage_x(0)
                    if C > 1:
                        stage_x(1)

                # ---- K,Q projection for this q-chunk ----
                ps_qk = pspj.tile([128, 512], F32, tag="pspj")
                for d in range(DC):
                    nc.tensor.matmul(
                        ps_qk[:], wkq_t[:, d, :], xT[:, d, 512 * c:512 * (c + 1)],
                        start=(d == 0), stop=(d == DC - 1))
                nc.vector.tensor_copy(qkALL[:, 512 * c:512 * (c + 1)], ps_qk[:])
                # duplicates across partition halves (SBUF->SBUF DMA)
                nc.scalar.dma_start(
                    QLK[0:64, 512 * c:512 * (c + 1)],
                    qkALL[64:128, 512 * c:512 * (c + 1)])
                odd_src = qkALL[0:64, 512 * c:512 * (c + 1)].rearrange(
                    "p (a b f) -> p a b f", b=2, f=128)[:, :, 1, :]
                nc.scalar.dma_start(
                    QLK[64:128, 256 * c:256 * (c + 1)].rearrange(
                        "p (a f) -> p a f", f=128),
                    odd_src)

                # ---- V projection, column-packed across chunk pairs ----
                # (only on even c; computes vT for chunks c and c+1 at
                # tile_position (0,0)/(0,64) concurrently)
                if (c % 2 == 0) if V_PACK else True:
                    two = V_PACK and (c + 1 < C)
                    ps_vt = pspj.tile([128, 512], F32, tag="pspj")
                    for d in range(DC):
                        nc.tensor.matmul(
                            ps_vt[0:64, :], wv_t[:, d, :],
                            xT[:, d, 512 * c:512 * (c + 1)],
                            start=(d == 0), stop=(d == DC - 1),
                            tile_position=(0, 0), skip_group_check=True)
                        if two:
                            nc.tensor.matmul(
                                ps_vt[64:128, :], wv_t[:, d, :],
                                xT[:, d, 512 * (c + 1):512 * (c + 2)],
                                start=(d == 0), stop=(d == DC - 1),
                                tile_position=(0, 64), skip_group_check=True)
                    nhalf = 2 if two else 1
                    vt_sb = sbw.tile([128, 512], BF16, tag="vt")
                    nc.vector.tensor_copy(vt_sb[0:64 * nhalf, :], ps_vt[0:64 * nhalf, :])
                    # v into [s, e] layout via the DMA xbar (zero PE cost);
                    # the xbar needs a contiguous dest, so stage then copy
                    # (bf16 SBUF->SBUF single-src runs in 4x DVE mode).
                    for h in range(nhalf):
                        vstage = sbw.tile([128, 4, E], BF16, tag="vst")
                        nc.scalar.dma_start_transpose(
                            vstage[:], vt_sb[64 * h:64 * h + 64, :])
                        nc.vector.tensor_copy(
                            v_aug[:, 4 * (c + h):4 * (c + h) + 4, 0:E], vstage[:])

                if 0 < c + 1 < C:
                    stage_x(c + 1)

            def attention(c):
                # ---- attention for q-chunk c ----
                # Two-deep software pipeline: emit scores_{j}/exp_{j}, then
                # PV_{j-2}.
                ps_o = pso.tile([E + 1, 512], F32, tag="pso")
                npair = 2 * c + 2
                pending = []

                def flush_pv(rec):
                    pj, ppt, pt0, pt1, pc0, pc1 = rec
                    nc.tensor.matmul(
                        ps_o[:, pc0:512], v_aug[:, pt0, 0:E + 1], ppt[:, pc0:512],
                        start=(pj == 0), stop=False)
                    nc.tensor.matmul(
                        ps_o[:, pc1:512], v_aug[:, pt1, 0:E + 1],
                        ppt[:, 512 + pc1:1024],
                        start=False, stop=(pj == npair - 1))

                for j in range(npair):
                    t0, t1 = 2 * j, 2 * j + 1
                    d0 = 128 * t0 - 512 * c
                    d1 = d0 + 128
                    c0, c1 = max(d0, 0), max(d1, 0)
                    full = (d1 < 0) or (c1 == 0)  # both tiles strictly below diag
                    offload = (OFFLOAD and full and c1 == 0
                               and (j % OFF_MOD == OFF_PHASE))
                    ps_pair = psst.tile([128, 1024], F32, tag="st")
                    nc.tensor.matmul(
                        ps_pair[:, c0:512],
                        qkALL[0:64, 128 * t0:128 * (t0 + 1)],
                        QLK[0:64, 512 * c + c0:512 * (c + 1)],
                        start=True, stop=True, tile_position=(0, 0))
                    nc.tensor.matmul(
                        ps_pair[:, 512 + c1:1024],
                        QLK[64:128, 128 * j:128 * (j + 1)],
                        qkALL[64:128, 512 * c + c1:512 * (c + 1)],
                        start=True, stop=True, tile_position=(64, 0))
                    if offload:
                        # DVE Schraudolph: one tensor_scalar, int16 out; the
                        # bit pattern is bf16 exp(s/8) to +-3%.
                        pq = ptq.tile([128, 1024], I16, tag="ptq")
                        nc.vector.tensor_scalar(
                            pq[:], ps_pair[:], A_TRICK, B_TRICK, MULT, ADD)
                        pt = pq[:].bitcast(BF16)
                    else:
                        ptt = ptp.tile([128, 1024], BF16, tag="pt")
                        pt = ptt[:]
                        if c1 == 0:
                            nc.scalar.activation(pt[:, 0:1024], ps_pair[:, 0:1024],
                                                 EXP, scale=0.125)
                        else:
                            # diagonal pair: two instructions, skipping the
                            # hole [512:512+c1) that the matmuls never wrote.
                            # (ACT has slack post DVE-offload; the extra
                            # per-instr overhead is free.)
                            nc.scalar.activation(pt[:, c0:512], ps_pair[:, c0:512],
                                                 EXP, scale=0.125)
                            nc.scalar.activation(pt[:, 512 + c1:1024],
                                                 ps_pair[:, 512 + c1:1024],
                                                 EXP, scale=0.125)
                        # zero the masked triangle of diagonal tiles on GpSimd
                        if 0 <= d0:
                            nc.gpsimd.affine_select(
                                out=pt[:, d0:d0 + 128],
                                in_=pt[:, d0:d0 + 128],
                                compare_op=mybir.AluOpType.is_ge, fill=0.0,
                                base=0, pattern=[[1, 128]], channel_multiplier=-1)
                        if 0 <= d1 < 512:
                            nc.gpsimd.affine_select(
                                out=pt[:, 512 + d1:512 + d1 + 128],
                                in_=pt[:, 512 + d1:512 + d1 + 128],
                                compare_op=mybir.AluOpType.is_ge, fill=0.0,
                                base=0, pattern=[[1, 128]], channel_multiplier=-1)
                    pending.append((j, pt, t0, t1, c0, c1))
                    if len(pending) > 3:
                        flush_pv(pending.pop(0))
                for rec in pending:
                    flush_pv(rec)
                return ps_o

            def epilogue(c, ps_o):
                # ---- epilogue: DMA-xbar transpose, normalize, store ----
                nc.vector.tensor_copy(ot_fix[0:E + 1, :], ps_o[:])
                tr_sb = sbw.tile([128, 4, 80], BF16, tag="tr")
                nc.scalar.dma_start_transpose(tr_sb[:], ot_fix[:])
                rec = sbw.tile([128, 4], F32, tag="rec")
                nc.vector.reciprocal(rec[:], tr_sb[:, :, E:E + 1])
                out_sb = sbw.tile([128, 4, E], F32, tag="osb")
                for m in range(4):
                    nc.vector.tensor_scalar_mul(
                        out_sb[:, m, :],
                        tr_sb[:, m, 0:E],
                        rec[:, m:m + 1])
                nc.sync.dma_start(
                    out[512 * c:512 * (c + 1), :].rearrange("(m p) e -> p m e", p=128),
                    out_sb[:])

            # epilogue(c) is emitted after prologue(c+1): the out-DMA then
            # ranks below the next chunk's x loads on the shared DMA path.
            for _rep in range(reps):
                prev = None
                for c in range(C):
                    prologue(c, first=(_rep == 0 and c == 0))
                    if prev is not None:
                        epilogue(*prev)
                    prev = (c, attention(c))
                epilogue(*prev)

    nc.compile()
    return nc


def _get_nc():
    if "nc" not in _cache:
        _cache["nc"] = _build()
    return _cache["nc"]


def kernel(x, W_Q, W_K, W_V):
    import ml_dtypes
    from concourse import bass_utils

    x_bf = np.ascontiguousarray(
        np.asarray(x, dtype=np.float32).astype(ml_dtypes.bfloat16))
    W_Q = np.ascontiguousarray(np.asarray(W_Q, dtype=np.float32))
    W_K = np.ascontiguousarray(np.asarray(W_K, dtype=np.float32))
    W_V = np.ascontiguousarray(np.asarray(W_V, dtype=np.float32))
    nc = _get_nc()
    in_maps = [
        {"x_bf16": x_bf[b], "W_Q": W_Q, "W_K": W_K, "W_V": W_V} for b in range(B)
    ]
    res = bass_utils.run_bass_kernel_spmd(nc, in_maps, core_ids=list(range(N_CORES)))
    return np.stack([res.results[b]["out"] for b in range(B)], axis=0)


# revision 53
# speedup vs baseline: 1.2918x; 1.2918x over previous
"""Causal single-head attention on 8 Trainium2 NeuronCores.

Problem: x[8, 4096, 512] @ W_{Q,K,V}[512, 64] -> causal softmax attention
-> out[8, 4096, 64].

Sharding: data-parallel over batch, one batch element per core (B == n_cores
== 8), QKV weights replicated. No collectives.

Per-core design (S=4096, D=512, E=64):
  - x is staged host-side as bf16 and enters SBUF PRE-TRANSPOSED through the
    DMA xbar transpose engine (dma_start_transpose), one [512,512] chunk at
    a time: the PE never spends a cycle transposing x (the previous design
    burned ~25k PE cycles + 14k DVE cycles on that).
  - Transposed score layout ST[k_par, q_free]; the softmax denominator
    falls out of the PV matmul via an appended ones-column on V
    (v_aug [k, 65] in a stride-66 bf16 layout -> row 64 of out.T
    accumulates sum_k P[k,q]).
  - bf16 operands for every matmul (1 PE cycle/row at ANY moving width,
    unlike f32r's N>=256 requirement); fp32 PSUM accumulation.
  - Scores matmuls contract over E=64, so two k-tiles are packed into
    the PE array quadrants (tile_position (0,0)/(64,0)) and run concurrently.
  - Softmax exp is split across TWO engines: ACT (table exp, bf16 out) and
    DVE (Schraudolph bit-trick: i16 = s*(128/ln2)/8 + B via one tensor_scalar
    with int16 output; the int16 bit pattern IS bf16 exp(s/8) to within
    +-3%). Offloaded pairs are full (non-diagonal) so no masking is needed
    and every PSUM element is freshly written; 1/3 of full pairs go to DVE.
    Accuracy: 6.3e-3 rel err end-to-end on HW (gate 2e-2); exp is
    unnormalized (no max subtraction; |s|/8 <= ~6).
  - vT->v and the epilogue outT->out transposes also go through the DMA
    xbar instead of the PE, issued on the ACT HWDGE ring (nc.scalar) so they
    never queue behind the 2us x chunk loads on the SP ring (doing them on
    the SP ring measured 13us SLOWER). The xbar needs contiguous dests:
    v stages through a [128,4,64] tile + 4x-mode DVE copy; the epilogue
    transposes a fixed [80,512] tile (rows 65..79 zero-padded once).
  - Causality: strictly-upper tile pairs are skipped, diagonal-crossing
    tiles get column-restricted matmuls/exp plus a GpSimd affine_select
    zeroing the 128x128 triangle of exp'd scores.

Measured (reps-slope estimator, median/trimmed-mean of 80 diffs; run-to-run
  spread ~+-8us): baseline(prev session) 138.4/145.0 us -> this design
  115.5-124.3 med across three samples (~118-124 center).
  The same estimator overstates the harness "HW exec time" by a per-rep
  pipeline-boundary cost (~10us on an empty kernel).

HW facts learned (probes in probe_pe.py / probe.py):
  - PE never reaches the 2.4GHz warm rate here: 128 back-to-back N=512 bf16
    matmuls measure 470ns each with per-matmul PSUM-slot rotation, 282ns in
    a single accumulation group (213ns would be warm); per-instruction
    semaphore/slot overhead ~70-190ns dominates, so FEWER/BIGGER PE
    instructions and DMA-xbar offload beat cycle-count micro-optimizations.
  - Column tiling with tile_position (0,64) (M=64) dies at runtime (xbus
    quadrant-3 HW bug), so the V projection stays un-packed and the
    4-way-packed denominator scheme is infeasible.
  - bf16 matmul->PSUM output is TRN3-only (bass assert + HW); N caps at 512
    fp32 (one PSUM bank) for every matmul.
  - x DMA streams at ~330GB/s (8MB in ~25us, fully overlapped).

Schedule (Tile scheduler is a greedy priority heap; priority == emission
order):
  - Inner loop software-pipelined THREE deep: scores_{j}/exp_{j} are
    emitted three pairs before PV_{j} so the in-order PE queue never blocks
    behind a PV that waits on an ACT/DVE exp. QLK duplication DMAs ride the
    ACT HWDGE ring so a chunk's first scores matmul is never queued behind
    bulk x loads.
  - PSUM: psst 3x[128,1024]=6 banks (scores/exp pipeline), pso 1 (PV
    accumulator), pspj 1 (projections; phase-split so prologue(c+1) never
    chains behind chunk c's attention).
  - Projections are emitted before the next chunk's x staging; epilogue(c)
    after prologue(c+1) so its out-DMA yields to x prefetches.
  - A 1-element exp at build start pre-loads the ACT exp table set.
"""

import sys

sys.path.insert(0, "/opt/trn_rl_repo")
sys.path.insert(0, "/root/.axon_site/_ro/trn_rl_repo")

import numpy as np

B, S, D, E = 8, 4096, 512, 64
N_CORES = 8

# Schraudolph int16/bf16 exp: i16 = floor(z * 128/ln2 + B); bits are bf16.
# A includes the softmax 1/sqrt(64). B centers the sawtooth (+0.5 if HW
# truncates, which it does in CoreSim; the 0.25-unit ambiguity vs
# round-to-nearest is a 0.14% weight bias either way).
A_TRICK = float((128.0 / np.log(2.0)) / 8.0)
B_TRICK = 16250.65
# offload full pair j of chunk c to the DVE trick when j % MOD == PHASE
OFF_MOD, OFF_PHASE = 2, 1
V_PACK = False      # column-packed V projection (tile_position (0,64))
OFFLOAD = True      # DVE Schraudolph exp offload

_cache = {}


def _build(S=S, reps=1):
    import concourse.bass as bass
    import concourse.mybir as mybir
    import concourse.tile as tile
    from concourse import bacc
    from concourse.masks import make_identity

    F32 = mybir.dt.float32
    F32R = mybir.dt.float32r
    BF16 = mybir.dt.bfloat16
    I16 = mybir.dt.int16
    U16 = mybir.dt.uint16
    EXP = mybir.ActivationFunctionType.Exp
    MULT = mybir.AluOpType.mult
    ADD = mybir.AluOpType.add

    T = S // 128   # 128-row seq tiles
    C = S // 512   # 512-col q chunks
    DC = D // 128  # contraction chunks

    nc = bacc.Bacc("TRN2", target_bir_lowering=False, debug=False,
                   num_devices=N_CORES)
    xbf = nc.dram_tensor("x_bf16", [S, D], BF16, kind="ExternalInput").ap()
    wq = nc.dram_tensor("W_Q", [D, E], F32, kind="ExternalInput").ap()
    wk = nc.dram_tensor("W_K", [D, E], F32, kind="ExternalInput").ap()
    wv = nc.dram_tensor("W_V", [D, E], F32, kind="ExternalInput").ap()
    out = nc.dram_tensor("out", [S, E], F32, kind="ExternalOutput").ap()

    with tile.TileContext(nc) as tc:
        from contextlib import ExitStack

        with ExitStack() as ctx:
            const = ctx.enter_context(tc.tile_pool(name="const", bufs=1))
            big = ctx.enter_context(tc.tile_pool(name="big", bufs=1))
            sbw = ctx.enter_context(tc.tile_pool(name="work", bufs=4))
            ptp = ctx.enter_context(tc.tile_pool(name="pt", bufs=10))
            ptq = ctx.enter_context(tc.tile_pool(name="ptq", bufs=6))
            # PSUM budget (8 banks): psst 3x[128,1024 f32]=6, pso 1,
            # pspj 1 (projections). x arrives pre-transposed via the DMA
            # xbar so no transpose-staging banks are needed.
            pspj = ctx.enter_context(tc.tile_pool(name="pspj", bufs=1, space="PSUM"))
            psst = ctx.enter_context(tc.tile_pool(name="psst", bufs=3, space="PSUM"))
            pso = ctx.enter_context(tc.tile_pool(name="pso", bufs=1, space="PSUM"))

            # ---------------- constants ----------------
            wstage = const.tile([128, DC, 2 * E], F32)
            nc.sync.dma_start(wstage[:, :, 0:E], wk.rearrange("(c p) e -> p c e", p=128))
            nc.sync.dma_start(wstage[:, :, E:2 * E], wq.rearrange("(c p) e -> p c e", p=128))
            wvstage = const.tile([128, DC, E], F32)
            nc.sync.dma_start(wvstage[:], wv.rearrange("(c p) e -> p c e", p=128))
            # weights to bf16; out rows of QK psum: 0:64 = kT, 64:128 = qT
            wkq_t = const.tile([128, DC, 2 * E], BF16)
            nc.vector.tensor_copy(wkq_t[:], wstage[:])
            wv_t = const.tile([128, DC, E], BF16)
            nc.vector.tensor_copy(wv_t[:], wvstage[:])

            ident_bf = const.tile([128, 128], BF16)
            make_identity(nc, ident_bf[:])

            ones_st = const.tile([128, T], BF16)
            nc.gpsimd.memset(ones_st[:], 1.0)
            # warm the ACT exp table set before the first real exp
            warm = const.tile([1, 1], F32)
            nc.scalar.activation(warm[:], ident_bf[0:1, 0:1], EXP)

            # ---------------- big SBUF residents ----------------
            xT = big.tile([128, DC, S], BF16)        # x transposed, d on partitions
            qkALL = big.tile([128, S], BF16)         # [0:64]=kT, [64:128]=qT
            QLK = big.tile([128, S], BF16)           # [0:64]=qT ; [64:128, 0:S//2]=kT odd tiles
            # v rows + ones col; row stride padded to E+2 (bf16 4B-align)
            v_aug = big.tile([128, T, E + 2], BF16)
            nc.vector.tensor_copy(v_aug[:, :, E:E + 1], ones_st[:])
            # epilogue staging: two fixed buffers alternated by chunk
            # parity (a single one would make epilogue(c)'s DVE copy wait on
            # epilogue(c-1)'s xbar-transpose read -- and the in-order DVE
            # queue head-of-line blocks the attention exps behind it).
            # Zero-pad rows E..79 once; copies only ever touch rows 0..E.
            ot_fix_a = big.tile([80, 512], BF16)
            ot_fix_b = big.tile([80, 512], BF16)
            ot_fix = (ot_fix_a, ot_fix_b)
            nc.gpsimd.memset(ot_fix_a[E:80, :], 0.0)
            nc.gpsimd.memset(ot_fix_b[E:80, :], 0.0)

            def stage_x(cc):
                # xT arrives pre-transposed straight from DRAM via the DMA
                # xbar (x is staged host-side as bf16). One transposing DMA
                # per chunk; zero PE/DVE/GPSIMD cost.
                nc.sync.dma_start_transpose(
                    xT[:, :, 512 * cc:512 * (cc + 1)],
                    xbf[512 * cc:512 * (cc + 1), :])

            def proj_steps(c):
                # Projection work for chunk c, split into steps that the
                # attention pair loop of chunk c-1 interleaves between a
                # pair's scores/exp and the exp-dependent PV flush: engine
                # queues execute in STATIC emission order, so stall-filling
                # work must be emitted exactly there.
                def s_qk():
                    ps_qk = pspj.tile([128, 512], F32, tag="pspj")
                    for d in range(DC):
                        nc.tensor.matmul(
                            ps_qk[:], wkq_t[:, d, :], xT[:, d, 512 * c:512 * (c + 1)],
                            start=(d == 0), stop=(d == DC - 1))
                    nc.vector.tensor_copy(qkALL[:, 512 * c:512 * (c + 1)], ps_qk[:])
                    # duplicates across partition halves (SBUF->SBUF DMA)
                    nc.scalar.dma_start(
                        QLK[0:64, 512 * c:512 * (c + 1)],
                        qkALL[64:128, 512 * c:512 * (c + 1)])
                    odd_src = qkALL[0:64, 512 * c:512 * (c + 1)].rearrange(
                        "p (a b f) -> p a b f", b=2, f=128)[:, :, 1, :]
                    nc.scalar.dma_start(
                        QLK[64:128, 256 * c:256 * (c + 1)].rearrange(
                            "p (a f) -> p a f", f=128),
                        odd_src)

                def s_v():
                    ps_vt = pspj.tile([128, 512], F32, tag="pspj")
                    for d in range(DC):
                        nc.tensor.matmul(
                            ps_vt[0:64, :], wv_t[:, d, :],
                            xT[:, d, 512 * c:512 * (c + 1)],
                            start=(d == 0), stop=(d == DC - 1),
                            tile_position=(0, 0), skip_group_check=True)
                    vt_sb = sbw.tile([128, 512], BF16, tag="vt")
                    nc.vector.tensor_copy(vt_sb[0:64, :], ps_vt[0:64, :])
                    # v into [s, e] layout via the DMA xbar (zero PE cost);
                    # contiguous staging tile + 4x-mode DVE copy.
                    vstage = sbw.tile([128, 4, E], BF16, tag="vst")
                    nc.scalar.dma_start_transpose(vstage[:], vt_sb[0:64, :])
                    nc.vector.tensor_copy(
                        v_aug[:, 4 * c:4 * c + 4, 0:E], vstage[:])

                def s_x():
                    if c + 1 < C:
                        stage_x(c + 1)

                return [s_qk, s_v, s_x]

            def attention(c, fillers):
                # ---- attention for q-chunk c ----
                # Three-deep software pipeline: emit scores_{j}/exp_{j},
                # then a next-chunk projection filler step, then PV_{j-3}.
                ps_o = pso.tile([E + 1, 512], F32, tag="pso")
                npair = 2 * c + 2
                pending = []

                def flush_pv(rec):
                    pj, ppt, pt0, pt1, pc0, pc1 = rec
                    nc.tensor.matmul(
                        ps_o[:, pc0:512], v_aug[:, pt0, 0:E + 1], ppt[:, pc0:512],
                        start=(pj == 0), stop=False)
                    nc.tensor.matmul(
                        ps_o[:, pc1:512], v_aug[:, pt1, 0:E + 1],
                        ppt[:, 512 + pc1:1024],
                        start=False, stop=(pj == npair - 1))

                for j in range(npair):
                    t0, t1 = 2 * j, 2 * j + 1
                    d0 = 128 * t0 - 512 * c
                    d1 = d0 + 128
                    c0, c1 = max(d0, 0), max(d1, 0)
                    full = (d1 < 0) or (c1 == 0)  # both tiles strictly below diag
                    offload = (OFFLOAD and full and c1 == 0
                               and (j % OFF_MOD == OFF_PHASE))
                    ps_pair = psst.tile([128, 1024], F32, tag="st")
                    nc.tensor.matmul(
                        ps_pair[:, c0:512],
                        qkALL[0:64, 128 * t0:128 * (t0 + 1)],
                        QLK[0:64, 512 * c + c0:512 * (c + 1)],
                        start=True, stop=True, tile_position=(0, 0))
                    nc.tensor.matmul(
                        ps_pair[:, 512 + c1:1024],
                        QLK[64:128, 128 * j:128 * (j + 1)],
                        qkALL[64:128, 512 * c + c1:512 * (c + 1)],
                        start=True, stop=True, tile_position=(64, 0))
                    if offload:
                        # DVE Schraudolph: one tensor_scalar, int16 out; the
                        # bit pattern is bf16 exp(s/8) to +-3%.
                        pq = ptq.tile([128, 1024], I16, tag="ptq")
                        nc.vector.tensor_scalar(
                            pq[:], ps_pair[:], A_TRICK, B_TRICK, MULT, ADD)
                        pt = pq[:].bitcast(BF16)
                    else:
                        ptt = ptp.tile([128, 1024], BF16, tag="pt")
                        pt = ptt[:]
                        if c1 == 0:
                            nc.scalar.activation(pt[:, 0:1024], ps_pair[:, 0:1024],
                                                 EXP, scale=0.125)
                        else:
                            # diagonal pair: two instructions, skipping the
                            # hole [512:512+c1) that the matmuls never wrote.
                            # (ACT has slack post DVE-offload; the extra
                            # per-instr overhead is free.)
                            nc.scalar.activation(pt[:, c0:512], ps_pair[:, c0:512],
                                                 EXP, scale=0.125)
                            nc.scalar.activation(pt[:, 512 + c1:1024],
                                                 ps_pair[:, 512 + c1:1024],
                                                 EXP, scale=0.125)
                        # zero the masked triangle of diagonal tiles on GpSimd
                        if 0 <= d0:
                            nc.gpsimd.affine_select(
                                out=pt[:, d0:d0 + 128],
                                in_=pt[:, d0:d0 + 128],
                                compare_op=mybir.AluOpType.is_ge, fill=0.0,
                                base=0, pattern=[[1, 128]], channel_multiplier=-1)
                        if 0 <= d1 < 512:
                            nc.gpsimd.affine_select(
                                out=pt[:, 512 + d1:512 + d1 + 128],
                                in_=pt[:, 512 + d1:512 + d1 + 128],
                                compare_op=mybir.AluOpType.is_ge, fill=0.0,
                                base=0, pattern=[[1, 128]], channel_multiplier=-1)
                    pending.append((j, pt, t0, t1, c0, c1))
                    if j >= 1 and fillers:
                        fillers.pop(0)()
                    if len(pending) > 3:
                        flush_pv(pending.pop(0))
                while fillers:
                    fillers.pop(0)()
                for rec in pending:
                    flush_pv(rec)
                return ps_o

            def epilogue(c, ps_o):
                # ---- epilogue: DMA-xbar transpose, normalize, store ----
                of = ot_fix[c % 2]
                nc.vector.tensor_copy(of[0:E + 1, :], ps_o[:])
                tr_sb = sbw.tile([128, 4, 80], BF16, tag="tr")
                nc.scalar.dma_start_transpose(tr_sb[:], of[:])
                rec = sbw.tile([128, 4], F32, tag="rec")
                nc.vector.reciprocal(rec[:], tr_sb[:, :, E:E + 1])
                out_sb = sbw.tile([128, 4, E], F32, tag="osb")
                for m in range(4):
                    nc.vector.tensor_scalar_mul(
                        out_sb[:, m, :],
                        tr_sb[:, m, 0:E],
                        rec[:, m:m + 1])
                nc.sync.dma_start(
                    out[512 * c:512 * (c + 1), :].rearrange("(m p) e -> p m e", p=128),
                    out_sb[:])

            for _rep in range(reps):
                stage_x(0)
                for st in proj_steps(0):
                    st()
                prev = None
                for c in range(C):
                    if prev is not None:
                        epilogue(*prev)
                    fillers = proj_steps(c + 1) if c + 1 < C else []
                    prev = (c, attention(c, fillers))
                epilogue(*prev)

    nc.compile()
    return nc


def _get_nc():
    if "nc" not in _cache:
        _cache["nc"] = _build()
    return _cache["nc"]


def kernel(x, W_Q, W_K, W_V):
    import ml_dtypes
    from concourse import bass_utils

    x_bf = np.ascontiguousarray(
        np.asarray(x, dtype=np.float32).astype(ml_dtypes.bfloat16))
    W_Q = np.ascontiguousarray(np.asarray(W_Q, dtype=np.float32))
    W_K = np.ascontiguousarray(np.asarray(W_K, dtype=np.float32))
    W_V = np.ascontiguousarray(np.asarray(W_V, dtype=np.float32))
    nc = _get_nc()
    in_maps = [
        {"x_bf16": x_bf[b], "W_Q": W_Q, "W_K": W_K, "W_V": W_V} for b in range(B)
    ]
    res = bass_utils.run_bass_kernel_spmd(nc, in_maps, core_ids=list(range(N_CORES)))
    return np.stack([res.results[b]["out"] for b in range(B)], axis=0)
